# revision 30
# baseline (speedup 1.0000x reference)
"""Trainium2 Bass kernel for nn_GNN_Model (gnn_message_passing).

Data-parallel over B=16384 (query,mv) pairs across 8 cores (2048 each).
Host converts the node table to fp16; per core, feature-major pipeline:
  batched indirect DMA gathers (2048 rows / instruction, 256B rows)
  -> fp16 PE transposes -> stationary-weight gate matmuls (f,i)
  -> ACT sigmoid(+bias) / tanh -> DVE products + pairwise add-tree
  (mean over 32 neighbors) -> query/mv tail (Wo, Wmv, W1, W2)
  feature-major, biases folded.
"""

import os

import numpy as np

import concourse.bass as bass
import concourse.mybir as mybir
import concourse.tile as tile
from concourse import bacc
from concourse.bass_utils import run_bass_kernel_spmd
from concourse.masks import make_identity

N = 500000
D = 128
H = 256
B = 16384
KN = 32
NCORES = 8
BC = B // NCORES          # 2048 rows per core
GROUP = 2048              # rows per gather group (= 16 chunks of 128)
NGR = (BC * KN) // GROUP  # 32 neighbor groups
NCH = BC * KN // 128      # 512 neighbor chunks
NCHT = NCH + 2 * (BC // 128)  # + 16 query chunks + 16 mv chunks = 544
FP16 = mybir.dt.float16
F32 = mybir.dt.float32
LAST_EXEC_NS = None
LAST_RES = None


def _build(b2_imm: float):
    nc = bacc.Bacc(None, target_bir_lowering=False)

    feats = nc.dram_tensor("feats", [N, D], FP16, kind="ExternalInput")
    idx = nc.dram_tensor("idx", [128, NCHT], mybir.dt.int32, kind="ExternalInput")
    w_names = ["wf", "wi", "wo", "wmva", "wmvb", "w1qa", "w1qb", "w1ma", "w1mb"]
    wt = {n: nc.dram_tensor(n, [128, 128], FP16, kind="ExternalInput") for n in w_names}
    wt["w2a"] = nc.dram_tensor("w2a", [128, 1], FP16, kind="ExternalInput")
    wt["w2b"] = nc.dram_tensor("w2b", [128, 1], FP16, kind="ExternalInput")
    b_names = ["bf", "bi", "bo", "b1a", "b1b"]
    bt = {n: nc.dram_tensor(n, [128, 1], F32, kind="ExternalInput") for n in b_names}
    out = nc.dram_tensor("out", [1, BC], F32, kind="ExternalOutput")

    SIG = mybir.ActivationFunctionType.Sigmoid
    TANH = mybir.ActivationFunctionType.Tanh
    RELU = mybir.ActivationFunctionType.Relu
    COPY = mybir.ActivationFunctionType.Copy
    MUL = mybir.AluOpType.mult
    ADD = mybir.AluOpType.add

    with tile.TileContext(nc) as tc:
        with (
            tc.tile_pool(name="const", bufs=1) as cp,
            tc.tile_pool(name="stage", bufs=2) as stp,
            tc.tile_pool(name="xt", bufs=2) as xtp,
            tc.tile_pool(name="gate", bufs=2) as gp,
            tc.tile_pool(name="ve", bufs=2) as vp,
            tc.tile_pool(name="xtps", bufs=2, space="PSUM") as xtpp,
            tc.tile_pool(name="gps", bufs=1, space="PSUM") as gpp,
        ):
            idx_t = cp.tile([128, NCHT], mybir.dt.int32)
            # q/mv index columns land first: the first gather round needs
            # only those, so it starts before the bulk index transfer ends
            nc.sync.dma_start(out=idx_t[:, NCH:NCHT], in_=idx[:, NCH:NCHT])
            nc.sync.dma_start(out=idx_t[:, 0:NCH], in_=idx[:, 0:NCH])
            ident = cp.tile([128, 128], FP16)
            make_identity(nc, ident[:])
            w = {}
            for n, dr in wt.items():
                w[n] = cp.tile([128, dr.shape[1]], FP16, tag=f"w_{n}", name=f"w_{n}")
                nc.sync.dma_start(out=w[n][:], in_=dr[:])
            bias = {}
            for n, dr in bt.items():
                bias[n] = cp.tile([128, 1], F32, tag=f"b_{n}", name=f"b_{n}")
                nc.sync.dma_start(out=bias[n][:], in_=dr[:])
            c_sum = cp.tile([128, BC], F32)      # per-pair sum over k (fp32)
            qt_sb = cp.tile([128, BC], FP16)     # query feats transposed
            mvt_sb = cp.tile([128, BC], FP16)

            def gather_group(ch0, nch=32):
                # HW indirect DMA consumes exactly one index per partition,
                # so each 128-row chunk is one instruction. 32 chunks share
                # one stage round to halve the WAR boundaries.
                stage = stp.tile([128, nch * 128], FP16, tag="stage")
                for j in range(nch):
                    nc.gpsimd.indirect_dma_start(
                        out=stage[:, j * 128:(j + 1) * 128],
                        out_offset=None,
                        in_=feats[:],
                        in_offset=bass.IndirectOffsetOnAxis(
                            ap=idx_t[:, ch0 + j:ch0 + j + 1], axis=0
                        ),
                    )
                return stage

            def transpose_group(stage, dest_sb):
                # stage [128, 2048] fp16 (16 row-chunks) -> dest_sb transposed
                for half in range(2):
                    xt_ps = xtpp.tile([128, 1024], FP16, tag="xtps")
                    for t8 in range(8):
                        ch = half * 8 + t8
                        nc.tensor.transpose(
                            xt_ps[:, t8 * 128:(t8 + 1) * 128],
                            stage[:, ch * 128:(ch + 1) * 128],
                            ident[:],
                        )
                    nc.vector.tensor_copy(
                        out=dest_sb[:, half * 1024:(half + 1) * 1024],
                        in_=xt_ps[:],
                    )

            # ---- query/mv gathers + transposes + output gates (early) ----
            o_sb = {}
            qmv_stage = gather_group(NCH, 32)
            for src_name, dest in (("q", qt_sb), ("mv", mvt_sb)):
                half = 0 if src_name == "q" else 1
                transpose_group(qmv_stage[:, half * GROUP:(half + 1) * GROUP],
                                dest)
                o_t = cp.tile([128, BC], FP16, tag=f"o_{src_name}",
                              name=f"o_{src_name}")
                for hh in range(2):
                    o_ps = gpp.tile([128, 1024], F32, tag="fps")
                    for s2 in range(2):
                        c0 = hh * 1024 + s2 * 512
                        nc.tensor.matmul(o_ps[:, s2 * 512:(s2 + 1) * 512],
                                         lhsT=w["wo"][:], rhs=dest[:, c0:c0 + 512],
                                         start=True, stop=True)
                    nc.scalar.activation(o_t[:, hh * 1024:(hh + 1) * 1024],
                                         o_ps[:], SIG, bias=bias["bo"][:])
                o_sb[src_name] = o_t

            # ---- tail tiles + per-column-half tail (runs early for half 0)
            tc_sb = cp.tile([128, BC], FP16)
            emb = {"q": cp.tile([128, BC], FP16, tag="emb_q", name="emb_q"),
                   "mv": cp.tile([128, BC], FP16, tag="emb_mv", name="emb_mv")}
            hid = [cp.tile([128, BC], FP16, tag=f"hid{h}", name=f"hid{h}")
                   for h in range(2)]
            ben_sb = cp.tile([1, BC], F32)

            def tail_piece(c0, wd):
                # one c0:c0+w column slice of the post-aggregation tail
                cols = slice(c0, c0 + wd)
                nc.scalar.activation(tc_sb[:, cols], c_sum[:, cols], TANH,
                                     scale=1.0 / KN)
                for src_name, src_t in (("q", qt_sb), ("mv", mvt_sb)):
                    h_sb = vp.tile([128, wd], FP16, tag="h")
                    nc.vector.tensor_tensor(out=h_sb[:],
                                            in0=o_sb[src_name][:, cols],
                                            in1=tc_sb[:, cols], op=MUL)
                    e_ps = gpp.tile([128, wd], F32, tag="ips")
                    nc.tensor.matmul(e_ps[:], lhsT=w["wmva"][:],
                                     rhs=src_t[:, cols],
                                     start=True, stop=False)
                    nc.tensor.matmul(e_ps[:], lhsT=w["wmvb"][:],
                                     rhs=h_sb[:], start=False, stop=True)
                    nc.scalar.activation(emb[src_name][:, cols], e_ps[:], COPY)
                for hh in range(2):
                    wq = w["w1qa"] if hh == 0 else w["w1qb"]
                    wm = w["w1ma"] if hh == 0 else w["w1mb"]
                    b1 = bias["b1a"] if hh == 0 else bias["b1b"]
                    h_ps = gpp.tile([128, wd], F32, tag="fps")
                    nc.tensor.matmul(h_ps[:], lhsT=wq[:],
                                     rhs=emb["q"][:, cols],
                                     start=True, stop=False)
                    nc.tensor.matmul(h_ps[:], lhsT=wm[:],
                                     rhs=emb["mv"][:, cols],
                                     start=False, stop=True)
                    nc.scalar.activation(hid[hh][:, cols], h_ps[:], RELU,
                                         bias=b1[:])
                b_ps = gpp.tile([1, wd], F32, tag="bps")
                nc.tensor.matmul(b_ps[:], lhsT=w["w2a"][:],
                                 rhs=hid[0][:, cols],
                                 start=True, stop=False)
                nc.tensor.matmul(b_ps[:], lhsT=w["w2b"][:],
                                 rhs=hid[1][:, cols],
                                 start=False, stop=True)
                nc.scalar.activation(ben_sb[:, cols], b_ps[:], COPY,
                                     bias=float(b2_imm))
                nc.sync.dma_start(out=out[:, cols], in_=ben_sb[:, cols])

            # ---- main neighbor loop ----
            for g in range(NGR):
                if g >= NGR - 2:
                    # final two groups gather in 16-chunk rounds so group 30's
                    # compute hides under group 31's gathers
                    stage = gather_group(g * 16, 16)
                else:
                    if g % 2 == 0:
                        stage2 = gather_group(g * 16, 32)
                    stage = stage2[:, (g % 2) * GROUP:(g % 2 + 1) * GROUP]
                xt_sb = xtp.tile([128, GROUP], FP16, tag="xt")
                transpose_group(stage, xt_sb)
                f_sb = gp.tile([128, GROUP], FP16, tag="f")
                i_sb = gp.tile([128, GROUP], FP16, tag="i")
                t_sb = gp.tile([128, GROUP], FP16, tag="t")
                for hh in range(2):  # halves of 1024 cols
                    f_ps = gpp.tile([128, 1024], F32, tag="fps")
                    i_ps = gpp.tile([128, 1024], F32, tag="ips")
                    for s2 in range(2):
                        rhs = xt_sb[:, (hh * 2 + s2) * 512:(hh * 2 + s2 + 1) * 512]
                        nc.tensor.matmul(f_ps[:, s2 * 512:(s2 + 1) * 512],
                                         lhsT=w["wf"][:], rhs=rhs, start=True, stop=True)
                    for s2 in range(2):
                        rhs = xt_sb[:, (hh * 2 + s2) * 512:(hh * 2 + s2 + 1) * 512]
                        nc.tensor.matmul(i_ps[:, s2 * 512:(s2 + 1) * 512],
                                         lhsT=w["wi"][:], rhs=rhs, start=True, stop=True)
                    nc.scalar.activation(f_sb[:, hh * 1024:(hh + 1) * 1024], f_ps[:],
                                         SIG, bias=bias["bf"][:])
                    nc.scalar.activation(i_sb[:, hh * 1024:(hh + 1) * 1024], i_ps[:],
                                         SIG, bias=bias["bi"][:])
                nc.scalar.activation(t_sb[:], xt_sb[:], TANH)
                fi = vp.tile([128, GROUP], FP16, tag="fi")
                prod = vp.tile([128, GROUP], FP16, tag="prod")
                nc.vector.tensor_tensor(out=fi[:], in0=f_sb[:], in1=i_sb[:], op=MUL)
                nc.vector.tensor_tensor(out=prod[:], in0=fi[:], in1=t_sb[:], op=MUL)
                # pairwise add-tree over k=32 (keeps DVE in 2x fp16 mode)
                a1 = vp.tile([128, 1024], FP16, tag="a1")
                a2 = vp.tile([128, 512], FP16, tag="a2")
                a3 = vp.tile([128, 256], FP16, tag="a3")
                a4 = vp.tile([128, 128], FP16, tag="a4")
                p3 = prod[:].rearrange("p (b k) -> p b k", k=32)
                nc.vector.tensor_tensor(out=a1[:], in0=p3[:, :, 0:16],
                                        in1=p3[:, :, 16:32], op=ADD)
                v1 = a1[:].rearrange("p (b k) -> p b k", k=16)
                nc.vector.tensor_tensor(out=a2[:], in0=v1[:, :, 0:8],
                                        in1=v1[:, :, 8:16], op=ADD)
                v2 = a2[:].rearrange("p (b k) -> p b k", k=8)
                nc.vector.tensor_tensor(out=a3[:], in0=v2[:, :, 0:4],
                                        in1=v2[:, :, 4:8], op=ADD)
                v3 = a3[:].rearrange("p (b k) -> p b k", k=4)
                nc.vector.tensor_tensor(out=a4[:], in0=v3[:, :, 0:2],
                                        in1=v3[:, :, 2:4], op=ADD)
                v4 = a4[:].rearrange("p (b k) -> p b k", k=2)
                nc.vector.tensor_tensor(
                    out=c_sum[:, g * (GROUP // KN):(g + 1) * (GROUP // KN)],
                    in0=v4[:, :, 0:1], in1=v4[:, :, 1:2], op=ADD)
                if g in (7, 15, 23):
                    # this slice of c_sum is complete: run its tail piece
                    # under the remaining groups' gathers
                    tail_piece((g // 8) * 512, 512)
                elif g == 30:
                    # all but group 31's columns: only a 64-col piece stays
                    # serial after the final gather round
                    tail_piece(1536, 448)

            tail_piece(1984, 64)

    nc.compile()
    return nc


def _chunk_idx(flat):
    # flat [n*128] -> [128, n] with idx[p, ch] = flat[ch*128 + p]
    return flat.reshape(-1, 128).T.copy()


def kernel(feats, query_idx, mv_idx, neighbor_idx,
           Wf, bf, Wi, bi, Wo, bo, Wmv, bmv, W1, b1, W2, b2):
    feats16 = np.ascontiguousarray(np.asarray(feats).astype(np.float16))
    query_idx = np.asarray(query_idx).astype(np.int32)
    mv_idx = np.asarray(mv_idx).astype(np.int32)
    neighbor_idx = np.asarray(neighbor_idx).astype(np.int32)
    Wf, Wi, Wo = [np.asarray(x, np.float32) for x in (Wf, Wi, Wo)]
    Wmv, W1, W2 = [np.asarray(x, np.float32) for x in (Wmv, W1, W2)]
    bf, bi, bo, bmv, b1, b2 = [np.asarray(x, np.float32) for x in (bf, bi, bo, bmv, b1, b2)]

    b1_eff = b1 + W1.T @ np.concatenate([bmv, bmv])
    f16 = np.float16
    weights = {
        "wf": Wf.astype(f16), "wi": Wi.astype(f16), "wo": Wo.astype(f16),
        "wmva": Wmv[0:128].astype(f16), "wmvb": Wmv[128:256].astype(f16),
        "w1qa": W1[0:128, 0:128].astype(f16), "w1qb": W1[0:128, 128:256].astype(f16),
        "w1ma": W1[128:256, 0:128].astype(f16), "w1mb": W1[128:256, 128:256].astype(f16),
        "w2a": np.ascontiguousarray(W2[0:128]).astype(f16),
        "w2b": np.ascontiguousarray(W2[128:256]).astype(f16),
    }
    biases = {
        "bf": bf.reshape(128, 1), "bi": bi.reshape(128, 1), "bo": bo.reshape(128, 1),
        "b1a": b1_eff[0:128].reshape(128, 1).astype(np.float32),
        "b1b": b1_eff[128:256].reshape(128, 1).astype(np.float32),
    }

    in_maps = []
    for c in range(NCORES):
        b0 = c * BC
        flat_nbr = neighbor_idx[b0:b0 + BC].reshape(-1)  # [BC*KN], b-major
        cols = [_chunk_idx(flat_nbr),
                _chunk_idx(query_idx[b0:b0 + BC]),
                _chunk_idx(mv_idx[b0:b0 + BC])]
        idx_all = np.concatenate(cols, axis=1).astype(np.int32)
        im = {"feats": feats16, "idx": np.ascontiguousarray(idx_all)}
        im.update(weights)
        im.update({k: np.ascontiguousarray(v) for k, v in biases.items()})
        in_maps.append(im)

    nc = _build(float(b2.reshape(-1)[0]))
    trace = bool(int(os.environ.get("KBENCH_TRACE", "0")))
    res = run_bass_kernel_spmd(nc, in_maps, core_ids=list(range(NCORES)), trace=trace)
    global LAST_EXEC_NS, LAST_RES
    LAST_EXEC_NS = res.exec_time_ns
    LAST_RES = res
    outp = np.empty((B, 1), dtype=np.float32)
    for c in range(NCORES):
        outp[c * BC:(c + 1) * BC, 0] = res.results[c]["out"][0]
    return outp
